# revision 25
# baseline (speedup 1.0000x reference)
"""Trainium2 Bass kernel for nn_CA_82300163326040.

Cross-attention between X and Y with softmax over the BATCH axis (torch
legacy dim=0). B=8, N=2048, D=512, f32.

Sharding: data-parallel over batch, one batch per NeuronCore (8 cores).
The batch-axis softmax couples cores: each core computes its local
exp-scores E1=exp(Q1.K2^T*s), E2=exp(Q2.K1^T*s) ([2048,2048]) and the
denominators Z = sum_b E are obtained with 8-core bf16 AllReduces.
No max-subtraction is needed: S ~ N(0, 1/9) so exp() cannot overflow.

Compute dtype: bf16 matmuls (PE full rate), f32 PSUM + softmax plumbing.
All DRAM layouts are pre-blocked on host so every device DMA moves large
contiguous chunks; E/Z are blocked [ch, mt, p, 512] (n-chunk major) so
phase-B writes one 128KB block per score tile and phase C stages whole
2MB chunks contiguously.

Per-core phases:
  A: six projections. Q/K transposed [e,n] via ACT-bias eviction; V
     row-major [m,e] via DVE bias add. 1/sqrt(D) folded into Q weights.
  B: T[m,n] = S^T tiles via bf16 matmuls, exp fused into PSUM eviction
     (ACT), E -> blocked DRAM on the gpsimd queue. AllReduce E -> Z.
  C: per n-chunk: stage Z/E halves, ACT-copy Z to f32, DVE fast-recip,
     A = E * rZ (mixed-dtype DVE mult -> bf16); U^T[d,n] accumulates
     V2^T A1 + V1^T A2 in PSUM (32 matmuls per [128,512] tile),
     eviction adds the (X+Y)^T residual. Output blocked, host unblocks.
"""

import numpy as np

import concourse.bass as bass
import concourse.mybir as mybir
import concourse.tile as tile
from concourse import bacc
from concourse.bass_utils import run_bass_kernel_spmd

P = 128
N = 2048  # sequence length
D = 512  # model dim
NCORES = 8
DT = D // P  # 4 feature tiles
NT = N // P  # 16 sequence tiles
CH = 512  # n-chunk (free dim of all matmuls)
NCH = N // CH  # 4 chunks
HT = NT // 2  # half of the m-tiles (staging granularity)

F32 = mybir.dt.float32
BF16 = mybir.dt.bfloat16
F8 = mybir.dt.float8e4

_CACHE = {}


def build(ar_dtype=BF16):
    nc = bacc.Bacc("TRN2", target_bir_lowering=False, debug=False, num_devices=NCORES)

    # ---- parameters (per core), all pre-arranged on host ----
    xtb = nc.declare_dram_parameter("XTB", [P, DT, N], BF16, isOutput=False)
    ytb = nc.declare_dram_parameter("YTB", [P, DT, N], BF16, isOutput=False)
    # combined residual (X+Y)^T, blocked [dt, ch, p, c], f32
    xyf = nc.declare_dram_parameter("XYF", [DT, NCH, P, CH], F32, isOutput=False)
    # weights (transposed, partition-major): w[p, o, e] = W^T[o*128+p, e]
    w_q1 = nc.declare_dram_parameter("WQ1T", [P, DT, D], BF16, isOutput=False)
    w_k1 = nc.declare_dram_parameter("WK1T", [P, DT, D], BF16, isOutput=False)
    w_v1 = nc.declare_dram_parameter("WV1T", [P, DT, D], BF16, isOutput=False)
    w_q2 = nc.declare_dram_parameter("WQ2T", [P, DT, D], BF16, isOutput=False)
    w_k2 = nc.declare_dram_parameter("WK2T", [P, DT, D], BF16, isOutput=False)
    w_v2 = nc.declare_dram_parameter("WV2T", [P, DT, D], BF16, isOutput=False)
    b_q1 = nc.declare_dram_parameter("BQ1", [P, DT], F32, isOutput=False)
    b_k1 = nc.declare_dram_parameter("BK1", [P, DT], F32, isOutput=False)
    b_q2 = nc.declare_dram_parameter("BQ2", [P, DT], F32, isOutput=False)
    b_k2 = nc.declare_dram_parameter("BK2", [P, DT], F32, isOutput=False)
    b_v1 = nc.declare_dram_parameter("BV1", [P, D], F32, isOutput=False)
    b_v2 = nc.declare_dram_parameter("BV2", [P, D], F32, isOutput=False)

    out = nc.declare_dram_parameter("OT", [DT, NCH, P, CH], F32, isOutput=True)

    with tile.TileContext(nc) as tc:
        with (
            tc.tile_pool(name="w", bufs=2) as p_w,
            tc.tile_pool(name="bias", bufs=1) as p_bias,
            tc.tile_pool(name="s2", bufs=6) as p_s2,
            tc.tile_pool(name="s1", bufs=6) as p_s1,
            tc.tile_pool(name="v", bufs=2) as p_v,
            tc.tile_pool(name="esb", bufs=8) as p_esb,
            tc.tile_pool(name="small", bufs=2) as p_small,
            tc.tile_pool(name="ps", bufs=8, space="PSUM") as p_ps,
            tc.tile_pool(name="dram", bufs=1, space="DRAM") as p_dram,
        ):
            # ---- DRAM intermediates, blocked [ch, mt, p, c] ----
            e1_d = p_dram.tile([NCH, NT, P, CH], ar_dtype, tag="e1")
            e2_d = p_dram.tile([NCH, NT, P, CH], ar_dtype, tag="e2")
            z1_d = p_dram.tile([NCH, NT, P, CH], ar_dtype, tag="z1",
                               addr_space="Shared")
            z2_a = p_dram.tile([3, NT, P, CH], ar_dtype, tag="z2a",
                               addr_space="Shared", name="z2a")
            z2_b = p_dram.tile([1, NT, P, CH], ar_dtype, tag="z2b",
                               addr_space="Shared", name="z2b")

            # ---- resident loads ----
            xt_sb = p_s2.tile([P, DT, N], BF16, tag="s2", name="xt")
            yt_sb = p_s2.tile([P, DT, N], BF16, tag="s2", name="yt")
            nc.sync.dma_start(xt_sb[:], xtb[:])
            nc.sync.dma_start(yt_sb[:], ytb[:])

            bq1_sb = p_bias.tile([P, DT], F32, tag="bq1")
            bk1_sb = p_bias.tile([P, DT], F32, tag="bk1")
            bq2_sb = p_bias.tile([P, DT], F32, tag="bq2")
            bk2_sb = p_bias.tile([P, DT], F32, tag="bk2")
            bv1_sb = p_bias.tile([P, D], F32, tag="bv1")
            bv2_sb = p_bias.tile([P, D], F32, tag="bv2")
            nc.sync.dma_start(bq1_sb[:], b_q1[:])
            nc.sync.dma_start(bk1_sb[:], b_k1[:])
            nc.sync.dma_start(bq2_sb[:], b_q2[:])
            nc.sync.dma_start(bk2_sb[:], b_k2[:])
            nc.sync.dma_start(bv1_sb[:], b_v1[:])
            nc.sync.dma_start(bv2_sb[:], b_v2[:])

            def load_w(wp):
                w_sb = p_w.tile([P, DT, D], BF16, tag="w")
                nc.sync.dma_start(w_sb[:], wp[:])
                return w_sb

            def proj_T(w_sb, src_sb, bias_sb, name):
                """out[e, n] = sum_d W[e,d] src[n,d] + b[e], e-major bf16."""
                o_sb = p_s2.tile([P, DT, N], BF16, tag="s2", name=name)
                for eo in range(DT):
                    for ch in range(NCH):
                        ps = p_ps.tile([P, CH], F32, tag="ps")
                        for do in range(DT):
                            nc.tensor.matmul(
                                ps[:],
                                w_sb[:, do, eo * P : (eo + 1) * P],
                                src_sb[:, do, ch * CH : (ch + 1) * CH],
                                start=(do == 0),
                                stop=(do == DT - 1),
                            )
                        nc.scalar.activation(
                            o_sb[:, eo, ch * CH : (ch + 1) * CH],
                            ps[:],
                            mybir.ActivationFunctionType.Identity,
                            bias=bias_sb[:, eo : eo + 1],
                        )
                return o_sb

            def proj_V(w_sb, src_sb, bias_sb, name):
                """out[m, e] = sum_d src[m,d] W[e,d] + b[e], m-major bf16."""
                o_sb = p_v.tile([P, NT, D], BF16, tag="v", name=name)
                for mt in range(NT):
                    ps = p_ps.tile([P, CH], F32, tag="ps")
                    for do in range(DT):
                        nc.tensor.matmul(
                            ps[:],
                            src_sb[:, do, mt * P : (mt + 1) * P],
                            w_sb[:, do, :],
                            start=(do == 0),
                            stop=(do == DT - 1),
                        )
                    nc.vector.tensor_add(out=o_sb[:, mt, :], in0=ps[:], in1=bias_sb[:])
                return o_sb

            def scores_exp(kt_sb, qt_sb, e_dram, chs):
                """E[ch, mt, p, c] = exp(sum_e K[m,e] Q[n,e]) -> blocked DRAM."""
                for ch in chs:
                    for mt in range(NT):
                        ps = p_ps.tile([P, CH], F32, tag="ps")
                        for eo in range(DT):
                            nc.tensor.matmul(
                                ps[:],
                                kt_sb[:, eo, mt * P : (mt + 1) * P],
                                qt_sb[:, eo, ch * CH : (ch + 1) * CH],
                                start=(eo == 0),
                                stop=(eo == DT - 1),
                            )
                        e_sb = p_esb.tile([P, CH], ar_dtype, tag="esb")
                        nc.scalar.activation(
                            e_sb[:], ps[:], mybir.ActivationFunctionType.Exp
                        )
                        nc.scalar.dma_start(e_dram[ch, mt], e_sb[:])

            def ar_full(e_d, z_d):
                nc.gpsimd.collective_compute(
                    "AllReduce",
                    mybir.AluOpType.add,
                    replica_groups=[list(range(NCORES))],
                    ins=[e_d.opt()],
                    outs=[z_d.opt()],
                )

            # ======== phase A1 + B1 + AR1 ========
            w_sb = load_w(w_q1)
            q1t = proj_T(w_sb, xt_sb, bq1_sb, "q1t")
            w_sb = load_w(w_k2)
            k2t = proj_T(w_sb, yt_sb, bk2_sb, "k2t")
            scores_exp(k2t, q1t, e1_d, range(NCH))
            ar_full(e1_d, z1_d)

            # ======== phase A2 + B2 + AR2 (split in halves) ========
            w_sb = load_w(w_k1)
            k1t = proj_T(w_sb, xt_sb, bk1_sb, "k1t")
            w_sb = load_w(w_q2)
            q2t = proj_T(w_sb, yt_sb, bq2_sb, "q2t")
            scores_exp(k1t, q2t, e2_d, (0, 1, 2))
            nc.gpsimd.collective_compute(
                "AllReduce",
                mybir.AluOpType.add,
                replica_groups=[list(range(NCORES))],
                ins=[e2_d[0:3].opt()],
                outs=[z2_a[:].opt()],
            )
            scores_exp(k1t, q2t, e2_d, (3,))
            nc.gpsimd.collective_compute(
                "AllReduce",
                mybir.AluOpType.add,
                replica_groups=[list(range(NCORES))],
                ins=[e2_d[3:4].opt()],
                outs=[z2_b[:].opt()],
            )

            # ======== phase A3 (V projections; fills the AR window) ========
            w_sb = load_w(w_v2)
            v2 = proj_V(w_sb, yt_sb, bv2_sb, "v2")
            w_sb = load_w(w_v1)
            v1 = proj_V(w_sb, xt_sb, bv1_sb, "v1")

            # ======== phase C ========
            # Split into a U1 pass (needs only Z1, runs under AllReduce #2)
            # and a U2 pass (after Z2). U1 partial sums are HELD OPEN in
            # PSUM banks (4 per chunk); the U2 matmuls append into the same
            # accumulation group. Emission order hand-interleaves chunks so
            # the in-order PE stream never parks behind a not-yet-ready
            # chunk while a ready one waits.
            def make_a_half(e_d, z_src, ch, h, name):
                """A[:, 8 mt, c] = E/Z for chunk ch, half h; bf16 [P, HT, CH]."""
                msl = slice(h * HT, (h + 1) * HT)
                e_b = p_s2.tile([P, HT, CH], ar_dtype, tag="s2", name=f"eb{name}")
                nc.sync.dma_start(e_b[:], e_d[ch, msl].rearrange("m p c -> p m c"))
                z_b = p_s2.tile([P, HT, CH], ar_dtype, tag="s2", name=f"zb{name}")
                nc.sync.dma_start(z_b[:], z_src(ch, msl))
                z_f = p_s2.tile([P, HT, CH], F32, tag="s2", name=f"zf{name}")
                nc.scalar.activation(
                    z_f[:], z_b[:], mybir.ActivationFunctionType.Copy
                )
                rz = p_s2.tile([P, HT, CH], F32, tag="s2", name=f"rz{name}")
                nc.vector.reciprocal_approx_fast(out=rz[:], in_=z_f[:])
                a_sb = p_s1.tile([P, HT, CH], BF16, tag="s1", name=f"a{name}")
                nc.vector.tensor_mul(out=a_sb[:], in0=e_b[:], in1=rz[:])
                return a_sb

            def z1_src(ch, msl):
                return z1_d[ch, msl].rearrange("m p c -> p m c")

            def z2_src(ch, msl):
                if ch < 3:
                    return z2_a[ch, msl].rearrange("m p c -> p m c")
                return z2_b[0, msl].rearrange("m p c -> p m c")

            ps_held = {}

            def u1_pass(ch):
                a1 = [make_a_half(e1_d, z1_src, ch, h, f"1{h}") for h in range(2)]
                tiles = []
                for dt in range(DT):
                    dsl = slice(dt * P, (dt + 1) * P)
                    ps = p_ps.tile([P, CH], F32, tag="ps")
                    for mt in range(NT):
                        nc.tensor.matmul(
                            ps[:], v2[:, mt, dsl], a1[mt // HT][:, mt % HT, :],
                            start=(mt == 0), stop=False,
                        )
                    tiles.append(ps)
                ps_held[ch] = tiles

            def u2_pass(ch):
                a2 = [make_a_half(e2_d, z2_src, ch, h, f"2{h}") for h in range(2)]
                for dt in range(DT):
                    dsl = slice(dt * P, (dt + 1) * P)
                    ps = ps_held[ch][dt]
                    for mt in range(NT):
                        nc.tensor.matmul(
                            ps[:], v1[:, mt, dsl], a2[mt // HT][:, mt % HT, :],
                            start=False, stop=(mt == NT - 1),
                        )
                    xyres = p_small.tile([P, CH], F32, tag="xyres")
                    nc.scalar.dma_start(xyres[:], xyf[dt, ch])
                    ot = p_small.tile([P, CH], F32, tag="ot")
                    nc.vector.tensor_add(out=ot[:], in0=ps[:], in1=xyres[:])
                    nc.scalar.dma_start(out[dt, ch], ot[:])

            u1_pass(0)
            u1_pass(1)
            u2_pass(0)
            u1_pass(2)
            u2_pass(1)
            u1_pass(3)
            u2_pass(2)
            u2_pass(3)

    nc.compile()
    return nc


def _pmajor(a, inner):
    """[O*P, F] -> [P, O, F] partition-major."""
    o = a.shape[0] // inner
    return np.ascontiguousarray(a.reshape(o, inner, a.shape[1]).transpose(1, 0, 2))


def _blocked(a):
    """[D, N] -> [DT, NCH, P, CH] blocked."""
    return np.ascontiguousarray(a.reshape(DT, P, NCH, CH).transpose(0, 2, 1, 3))


def _prep_inputs(inputs):
    import ml_dtypes

    X = np.asarray(inputs["X"], dtype=np.float32)
    Y = np.asarray(inputs["Y"], dtype=np.float32)
    scale = np.float32(1.0 / np.sqrt(D))

    def wT(name, s=np.float32(1.0)):
        w = np.asarray(inputs[f"W_{name}"], dtype=np.float32)
        return _pmajor((w.T * s).astype(ml_dtypes.bfloat16), P)

    def bstripe(name, s=np.float32(1.0)):
        b = np.asarray(inputs[f"b_{name}"], dtype=np.float32) * s
        return np.ascontiguousarray(b.reshape(DT, P).T)

    def bbcast(name):
        b = np.asarray(inputs[f"b_{name}"], dtype=np.float32)
        return np.ascontiguousarray(np.broadcast_to(b, (P, D)))

    shared = {
        "WQ1T": wT("xq", scale),
        "WK1T": wT("xk"),
        "WV1T": wT("xv"),
        "WQ2T": wT("yq", scale),
        "WK2T": wT("yk"),
        "WV2T": wT("yv"),
        "BQ1": bstripe("xq", scale),
        "BK1": bstripe("xk"),
        "BQ2": bstripe("yq", scale),
        "BK2": bstripe("yk"),
        "BV1": bbcast("xv"),
        "BV2": bbcast("yv"),
    }
    in_maps = []
    for c in range(NCORES):
        xt = np.ascontiguousarray(X[c].T)
        yt = np.ascontiguousarray(Y[c].T)
        m = dict(shared)
        m["XYF"] = _blocked(xt + yt)
        m["XTB"] = _pmajor(xt.astype(ml_dtypes.bfloat16), P)
        m["YTB"] = _pmajor(yt.astype(ml_dtypes.bfloat16), P)
        in_maps.append(m)
    return in_maps


def _unblock(ot):
    """[DT, NCH, P, CH] -> [N, D] (transposed back)."""
    return ot.transpose(0, 2, 1, 3).reshape(D, N).T


def kernel(**inputs):
    if "nc" not in _CACHE:
        _CACHE["nc"] = build()
    nc = _CACHE["nc"]
    in_maps = _prep_inputs(inputs)
    res = run_bass_kernel_spmd(
        nc, in_maps, core_ids=list(range(NCORES)), **_CACHE.get("run_kwargs", {})
    )
    _CACHE["last_result"] = res
    out = np.stack(
        [np.ascontiguousarray(_unblock(res.results[c]["OT"])) for c in range(NCORES)]
    )
    return out.astype(np.float32)


# revision 26
# speedup vs baseline: 1.0129x; 1.0129x over previous
"""Trainium2 Bass kernel for nn_CA_82300163326040.

Cross-attention between X and Y with softmax over the BATCH axis (torch
legacy dim=0). B=8, N=2048, D=512, f32.

Sharding: data-parallel over batch, one batch per NeuronCore (8 cores).
The batch-axis softmax couples cores: each core computes its local
exp-scores E1=exp(Q1.K2^T*s), E2=exp(Q2.K1^T*s) ([2048,2048]) and the
denominators Z = sum_b E are obtained with 8-core bf16 AllReduces.
No max-subtraction is needed: S ~ N(0, 1/9) so exp() cannot overflow.

Compute dtype: bf16 matmuls (PE full rate), f32 PSUM + softmax plumbing.
All DRAM layouts are pre-blocked on host so every device DMA moves large
contiguous chunks; E/Z are blocked [ch, mt, p, 512] (n-chunk major) so
phase-B writes one 128KB block per score tile and phase C stages whole
2MB chunks contiguously.

Per-core phases:
  A: six projections. Q/K transposed [e,n] via ACT-bias eviction; V
     row-major [m,e] via DVE bias add. 1/sqrt(D) folded into Q weights.
  B: T[m,n] = S^T tiles via bf16 matmuls, exp fused into PSUM eviction
     (ACT), E -> blocked DRAM on the gpsimd queue. AllReduce E -> Z.
  C: per n-chunk: stage Z/E halves, ACT-copy Z to f32, DVE fast-recip,
     A = E * rZ (mixed-dtype DVE mult -> bf16); U^T[d,n] accumulates
     V2^T A1 + V1^T A2 in PSUM (32 matmuls per [128,512] tile),
     eviction adds the (X+Y)^T residual. Output blocked, host unblocks.
"""

import numpy as np

import concourse.bass as bass
import concourse.mybir as mybir
import concourse.tile as tile
from concourse import bacc
from concourse.bass_utils import run_bass_kernel_spmd

P = 128
N = 2048  # sequence length
D = 512  # model dim
NCORES = 8
DT = D // P  # 4 feature tiles
NT = N // P  # 16 sequence tiles
CH = 512  # n-chunk (free dim of all matmuls)
NCH = N // CH  # 4 chunks
HT = NT // 2  # half of the m-tiles (staging granularity)

F32 = mybir.dt.float32
BF16 = mybir.dt.bfloat16
F8 = mybir.dt.float8e4

_CACHE = {}


def build(ar_dtype=BF16):
    nc = bacc.Bacc("TRN2", target_bir_lowering=False, debug=False, num_devices=NCORES)

    # ---- parameters (per core), all pre-arranged on host ----
    xtb = nc.declare_dram_parameter("XTB", [P, DT, N], BF16, isOutput=False)
    ytb = nc.declare_dram_parameter("YTB", [P, DT, N], BF16, isOutput=False)
    # combined residual (X+Y)^T, blocked [dt, ch, p, c], f32
    xyf = nc.declare_dram_parameter("XYF", [DT, NCH, P, CH], F32, isOutput=False)
    # weights (transposed, partition-major): w[p, o, e] = W^T[o*128+p, e]
    w_q1 = nc.declare_dram_parameter("WQ1T", [P, DT, D], BF16, isOutput=False)
    w_k1 = nc.declare_dram_parameter("WK1T", [P, DT, D], BF16, isOutput=False)
    w_v1 = nc.declare_dram_parameter("WV1T", [P, DT, D], BF16, isOutput=False)
    w_q2 = nc.declare_dram_parameter("WQ2T", [P, DT, D], BF16, isOutput=False)
    w_k2 = nc.declare_dram_parameter("WK2T", [P, DT, D], BF16, isOutput=False)
    w_v2 = nc.declare_dram_parameter("WV2T", [P, DT, D], BF16, isOutput=False)
    b_q1 = nc.declare_dram_parameter("BQ1", [P, DT], F32, isOutput=False)
    b_k1 = nc.declare_dram_parameter("BK1", [P, DT], F32, isOutput=False)
    b_q2 = nc.declare_dram_parameter("BQ2", [P, DT], F32, isOutput=False)
    b_k2 = nc.declare_dram_parameter("BK2", [P, DT], F32, isOutput=False)
    b_v1 = nc.declare_dram_parameter("BV1", [P, D], F32, isOutput=False)
    b_v2 = nc.declare_dram_parameter("BV2", [P, D], F32, isOutput=False)

    out = nc.declare_dram_parameter("OT", [DT, NCH, P, CH], F32, isOutput=True)

    with tile.TileContext(nc) as tc:
        with (
            tc.tile_pool(name="w", bufs=2) as p_w,
            tc.tile_pool(name="bias", bufs=1) as p_bias,
            tc.tile_pool(name="s2", bufs=6) as p_s2,
            tc.tile_pool(name="s1", bufs=6) as p_s1,
            tc.tile_pool(name="v", bufs=2) as p_v,
            tc.tile_pool(name="esb", bufs=8) as p_esb,
            tc.tile_pool(name="small", bufs=2) as p_small,
            tc.tile_pool(name="ps", bufs=8, space="PSUM") as p_ps,
            tc.tile_pool(name="dram", bufs=1, space="DRAM") as p_dram,
        ):
            # ---- DRAM intermediates, blocked [ch, mt, p, c] ----
            e1_d = p_dram.tile([NCH, NT, P, CH], ar_dtype, tag="e1")
            e2_d = p_dram.tile([NCH, NT, P, CH], ar_dtype, tag="e2")
            z1_d = p_dram.tile([NCH, NT, P, CH], ar_dtype, tag="z1",
                               addr_space="Shared")
            z2_d = p_dram.tile([NCH, NT, P, CH], ar_dtype, tag="z2",
                               addr_space="Shared")

            # ---- resident loads ----
            xt_sb = p_s2.tile([P, DT, N], BF16, tag="s2", name="xt")
            yt_sb = p_s2.tile([P, DT, N], BF16, tag="s2", name="yt")
            nc.sync.dma_start(xt_sb[:], xtb[:])
            nc.sync.dma_start(yt_sb[:], ytb[:])

            bq1_sb = p_bias.tile([P, DT], F32, tag="bq1")
            bk1_sb = p_bias.tile([P, DT], F32, tag="bk1")
            bq2_sb = p_bias.tile([P, DT], F32, tag="bq2")
            bk2_sb = p_bias.tile([P, DT], F32, tag="bk2")
            bv1_sb = p_bias.tile([P, D], F32, tag="bv1")
            bv2_sb = p_bias.tile([P, D], F32, tag="bv2")
            nc.sync.dma_start(bq1_sb[:], b_q1[:])
            nc.sync.dma_start(bk1_sb[:], b_k1[:])
            nc.sync.dma_start(bq2_sb[:], b_q2[:])
            nc.sync.dma_start(bk2_sb[:], b_k2[:])
            nc.sync.dma_start(bv1_sb[:], b_v1[:])
            nc.sync.dma_start(bv2_sb[:], b_v2[:])

            def load_w(wp):
                w_sb = p_w.tile([P, DT, D], BF16, tag="w")
                nc.sync.dma_start(w_sb[:], wp[:])
                return w_sb

            def proj_T(w_sb, src_sb, bias_sb, name):
                """out[e, n] = sum_d W[e,d] src[n,d] + b[e], e-major bf16."""
                o_sb = p_s2.tile([P, DT, N], BF16, tag="s2", name=name)
                for eo in range(DT):
                    for ch in range(NCH):
                        ps = p_ps.tile([P, CH], F32, tag="ps")
                        for do in range(DT):
                            nc.tensor.matmul(
                                ps[:],
                                w_sb[:, do, eo * P : (eo + 1) * P],
                                src_sb[:, do, ch * CH : (ch + 1) * CH],
                                start=(do == 0),
                                stop=(do == DT - 1),
                            )
                        nc.scalar.activation(
                            o_sb[:, eo, ch * CH : (ch + 1) * CH],
                            ps[:],
                            mybir.ActivationFunctionType.Identity,
                            bias=bias_sb[:, eo : eo + 1],
                        )
                return o_sb

            def proj_V(w_sb, src_sb, bias_sb, name):
                """out[m, e] = sum_d src[m,d] W[e,d] + b[e], m-major bf16."""
                o_sb = p_v.tile([P, NT, D], BF16, tag="v", name=name)
                for mt in range(NT):
                    ps = p_ps.tile([P, CH], F32, tag="ps")
                    for do in range(DT):
                        nc.tensor.matmul(
                            ps[:],
                            src_sb[:, do, mt * P : (mt + 1) * P],
                            w_sb[:, do, :],
                            start=(do == 0),
                            stop=(do == DT - 1),
                        )
                    nc.vector.tensor_add(out=o_sb[:, mt, :], in0=ps[:], in1=bias_sb[:])
                return o_sb

            def scores_exp(kt_sb, qt_sb, e_dram, chs):
                """E[ch, mt, p, c] = exp(sum_e K[m,e] Q[n,e]) -> blocked DRAM."""
                for ch in chs:
                    for mt in range(NT):
                        ps = p_ps.tile([P, CH], F32, tag="ps")
                        for eo in range(DT):
                            nc.tensor.matmul(
                                ps[:],
                                kt_sb[:, eo, mt * P : (mt + 1) * P],
                                qt_sb[:, eo, ch * CH : (ch + 1) * CH],
                                start=(eo == 0),
                                stop=(eo == DT - 1),
                            )
                        e_sb = p_esb.tile([P, CH], ar_dtype, tag="esb")
                        nc.scalar.activation(
                            e_sb[:], ps[:], mybir.ActivationFunctionType.Exp
                        )
                        nc.scalar.dma_start(e_dram[ch, mt], e_sb[:])

            def ar_full(e_d, z_d):
                nc.gpsimd.collective_compute(
                    "AllReduce",
                    mybir.AluOpType.add,
                    replica_groups=[list(range(NCORES))],
                    ins=[e_d.opt()],
                    outs=[z_d.opt()],
                )

            # ======== phase A1 + B1 + AR1 ========
            w_sb = load_w(w_q1)
            q1t = proj_T(w_sb, xt_sb, bq1_sb, "q1t")
            w_sb = load_w(w_k2)
            k2t = proj_T(w_sb, yt_sb, bk2_sb, "k2t")
            scores_exp(k2t, q1t, e1_d, range(NCH))
            ar_full(e1_d, z1_d)

            # ======== phase A2 + B2 + AR2 (split in halves) ========
            w_sb = load_w(w_k1)
            k1t = proj_T(w_sb, xt_sb, bk1_sb, "k1t")
            w_sb = load_w(w_q2)
            q2t = proj_T(w_sb, yt_sb, bq2_sb, "q2t")
            scores_exp(k1t, q2t, e2_d, range(NCH))
            ar_full(e2_d, z2_d)

            # ======== phase A3 (V projections; fills the AR window) ========
            w_sb = load_w(w_v2)
            v2 = proj_V(w_sb, yt_sb, bv2_sb, "v2")
            w_sb = load_w(w_v1)
            v1 = proj_V(w_sb, xt_sb, bv1_sb, "v1")

            # ======== phase C ========
            # Split into a U1 pass (needs only Z1, runs under AllReduce #2)
            # and a U2 pass (after Z2). U1 partial sums are HELD OPEN in
            # PSUM banks (4 per chunk); the U2 matmuls append into the same
            # accumulation group. Emission order hand-interleaves chunks so
            # the in-order PE stream never parks behind a not-yet-ready
            # chunk while a ready one waits.
            def make_a_half(e_d, z_src, ch, h, name, mult_eng=None):
                """A[:, 8 mt, c] = E/Z for chunk ch, half h; bf16 [P, HT, CH]."""
                msl = slice(h * HT, (h + 1) * HT)
                e_b = p_s2.tile([P, HT, CH], ar_dtype, tag="s2", name=f"eb{name}")
                nc.sync.dma_start(e_b[:], e_d[ch, msl].rearrange("m p c -> p m c"))
                z_b = p_s2.tile([P, HT, CH], ar_dtype, tag="s2", name=f"zb{name}")
                nc.sync.dma_start(z_b[:], z_src(ch, msl))
                z_f = p_s2.tile([P, HT, CH], F32, tag="s2", name=f"zf{name}")
                nc.scalar.activation(
                    z_f[:], z_b[:], mybir.ActivationFunctionType.Copy
                )
                rz = p_s2.tile([P, HT, CH], F32, tag="s2", name=f"rz{name}")
                nc.vector.reciprocal_approx_fast(out=rz[:], in_=z_f[:])
                a_sb = p_s1.tile([P, HT, CH], BF16, tag="s1", name=f"a{name}")
                (mult_eng or nc.vector).tensor_mul(
                    out=a_sb[:], in0=e_b[:], in1=rz[:]
                )
                return a_sb

            def z1_src(ch, msl):
                return z1_d[ch, msl].rearrange("m p c -> p m c")

            def z2_src(ch, msl):
                return z2_d[ch, msl].rearrange("m p c -> p m c")

            ps_held = {}

            def u1_pass(ch):
                a1 = [make_a_half(e1_d, z1_src, ch, h, f"1{h}") for h in range(2)]
                tiles = []
                for dt in range(DT):
                    dsl = slice(dt * P, (dt + 1) * P)
                    ps = p_ps.tile([P, CH], F32, tag="ps")
                    for mt in range(NT):
                        nc.tensor.matmul(
                            ps[:], v2[:, mt, dsl], a1[mt // HT][:, mt % HT, :],
                            start=(mt == 0), stop=False,
                        )
                    tiles.append(ps)
                ps_held[ch] = tiles

            def u2_pass(ch):
                eng = nc.vector if ch % 2 == 0 else nc.gpsimd
                a2 = [make_a_half(e2_d, z2_src, ch, h, f"2{h}", eng)
                      for h in range(2)]
                for dt in range(DT):
                    dsl = slice(dt * P, (dt + 1) * P)
                    ps = ps_held[ch][dt]
                    for mt in range(NT):
                        nc.tensor.matmul(
                            ps[:], v1[:, mt, dsl], a2[mt // HT][:, mt % HT, :],
                            start=False, stop=(mt == NT - 1),
                        )
                    xyres = p_small.tile([P, CH], F32, tag="xyres")
                    nc.scalar.dma_start(xyres[:], xyf[dt, ch])
                    ot = p_small.tile([P, CH], F32, tag="ot")
                    nc.vector.tensor_add(out=ot[:], in0=ps[:], in1=xyres[:])
                    nc.scalar.dma_start(out[dt, ch], ot[:])

            u1_pass(0)
            u1_pass(1)
            u2_pass(0)
            u1_pass(2)
            u2_pass(1)
            u1_pass(3)
            u2_pass(2)
            u2_pass(3)

    nc.compile()
    return nc


def _pmajor(a, inner):
    """[O*P, F] -> [P, O, F] partition-major."""
    o = a.shape[0] // inner
    return np.ascontiguousarray(a.reshape(o, inner, a.shape[1]).transpose(1, 0, 2))


def _blocked(a):
    """[D, N] -> [DT, NCH, P, CH] blocked."""
    return np.ascontiguousarray(a.reshape(DT, P, NCH, CH).transpose(0, 2, 1, 3))


def _prep_inputs(inputs):
    import ml_dtypes

    X = np.asarray(inputs["X"], dtype=np.float32)
    Y = np.asarray(inputs["Y"], dtype=np.float32)
    scale = np.float32(1.0 / np.sqrt(D))

    def wT(name, s=np.float32(1.0)):
        w = np.asarray(inputs[f"W_{name}"], dtype=np.float32)
        return _pmajor((w.T * s).astype(ml_dtypes.bfloat16), P)

    def bstripe(name, s=np.float32(1.0)):
        b = np.asarray(inputs[f"b_{name}"], dtype=np.float32) * s
        return np.ascontiguousarray(b.reshape(DT, P).T)

    def bbcast(name):
        b = np.asarray(inputs[f"b_{name}"], dtype=np.float32)
        return np.ascontiguousarray(np.broadcast_to(b, (P, D)))

    shared = {
        "WQ1T": wT("xq", scale),
        "WK1T": wT("xk"),
        "WV1T": wT("xv"),
        "WQ2T": wT("yq", scale),
        "WK2T": wT("yk"),
        "WV2T": wT("yv"),
        "BQ1": bstripe("xq", scale),
        "BK1": bstripe("xk"),
        "BQ2": bstripe("yq", scale),
        "BK2": bstripe("yk"),
        "BV1": bbcast("xv"),
        "BV2": bbcast("yv"),
    }
    in_maps = []
    for c in range(NCORES):
        xt = np.ascontiguousarray(X[c].T)
        yt = np.ascontiguousarray(Y[c].T)
        m = dict(shared)
        m["XYF"] = _blocked(xt + yt)
        m["XTB"] = _pmajor(xt.astype(ml_dtypes.bfloat16), P)
        m["YTB"] = _pmajor(yt.astype(ml_dtypes.bfloat16), P)
        in_maps.append(m)
    return in_maps


def _unblock(ot):
    """[DT, NCH, P, CH] -> [N, D] (transposed back)."""
    return ot.transpose(0, 2, 1, 3).reshape(D, N).T


def kernel(**inputs):
    if "nc" not in _CACHE:
        _CACHE["nc"] = build()
    nc = _CACHE["nc"]
    in_maps = _prep_inputs(inputs)
    res = run_bass_kernel_spmd(
        nc, in_maps, core_ids=list(range(NCORES)), **_CACHE.get("run_kwargs", {})
    )
    _CACHE["last_result"] = res
    out = np.stack(
        [np.ascontiguousarray(_unblock(res.results[c]["OT"])) for c in range(NCORES)]
    )
    return out.astype(np.float32)


# revision 27
# speedup vs baseline: 1.0575x; 1.0440x over previous
"""Trainium2 Bass kernel for nn_CA_82300163326040.

Cross-attention between X and Y with softmax over the BATCH axis (torch
legacy dim=0). B=8, N=2048, D=512, f32.

Sharding: data-parallel over batch, one batch per NeuronCore (8 cores).
The batch-axis softmax couples cores: each core computes its local
exp-scores E1=exp(Q1.K2^T*s), E2=exp(Q2.K1^T*s) ([2048,2048]) and the
denominators Z = sum_b E are obtained with 8-core bf16 AllReduces.
No max-subtraction is needed: S ~ N(0, 1/9) so exp() cannot overflow.

Compute dtype: bf16 matmuls (PE full rate), f32 PSUM + softmax plumbing.
All DRAM layouts are pre-blocked on host so every device DMA moves large
contiguous chunks; E/Z are blocked [ch, mt, p, 512] (n-chunk major) so
phase-B writes one 128KB block per score tile and phase C stages whole
2MB chunks contiguously.

Per-core phases:
  A: six projections. Q/K transposed [e,n] via ACT-bias eviction; V
     row-major [m,e] via DVE bias add. 1/sqrt(D) folded into Q weights.
  B: T[m,n] = S^T tiles via bf16 matmuls, exp fused into PSUM eviction
     (ACT), E -> blocked DRAM on the gpsimd queue. AllReduce E -> Z.
  C: per n-chunk: stage Z/E halves, ACT-copy Z to f32, DVE fast-recip,
     A = E * rZ (mixed-dtype DVE mult -> bf16); U^T[d,n] accumulates
     V2^T A1 + V1^T A2 in PSUM (32 matmuls per [128,512] tile),
     eviction adds the (X+Y)^T residual. Output blocked, host unblocks.
"""

import numpy as np

import concourse.bass as bass
import concourse.mybir as mybir
import concourse.tile as tile
from concourse import bacc
from concourse.bass_utils import run_bass_kernel_spmd

P = 128
N = 2048  # sequence length
D = 512  # model dim
NCORES = 8
DT = D // P  # 4 feature tiles
NT = N // P  # 16 sequence tiles
CH = 512  # n-chunk (free dim of all matmuls)
NCH = N // CH  # 4 chunks
HT = NT // 2  # half of the m-tiles (staging granularity)

F32 = mybir.dt.float32
BF16 = mybir.dt.bfloat16
F8 = mybir.dt.float8e4

_CACHE = {}


def build(ar_dtype=BF16):
    nc = bacc.Bacc("TRN2", target_bir_lowering=False, debug=False, num_devices=NCORES)

    # ---- parameters (per core), all pre-arranged on host ----
    xtb = nc.declare_dram_parameter("XTB", [P, DT, N], BF16, isOutput=False)
    ytb = nc.declare_dram_parameter("YTB", [P, DT, N], BF16, isOutput=False)
    # combined residual (X+Y)^T, blocked [dt, ch, p, c], f32
    xyf = nc.declare_dram_parameter("XYF", [DT, NCH, P, CH], F32, isOutput=False)
    # weights (transposed, partition-major): w[p, o, e] = W^T[o*128+p, e]
    w_q1 = nc.declare_dram_parameter("WQ1T", [P, DT, D], BF16, isOutput=False)
    w_k1 = nc.declare_dram_parameter("WK1T", [P, DT, D], BF16, isOutput=False)
    w_v1 = nc.declare_dram_parameter("WV1T", [P, DT, D], BF16, isOutput=False)
    w_q2 = nc.declare_dram_parameter("WQ2T", [P, DT, D], BF16, isOutput=False)
    w_k2 = nc.declare_dram_parameter("WK2T", [P, DT, D], BF16, isOutput=False)
    w_v2 = nc.declare_dram_parameter("WV2T", [P, DT, D], BF16, isOutput=False)
    b_q1 = nc.declare_dram_parameter("BQ1", [P, DT], F32, isOutput=False)
    b_k1 = nc.declare_dram_parameter("BK1", [P, DT], F32, isOutput=False)
    b_q2 = nc.declare_dram_parameter("BQ2", [P, DT], F32, isOutput=False)
    b_k2 = nc.declare_dram_parameter("BK2", [P, DT], F32, isOutput=False)
    b_v1 = nc.declare_dram_parameter("BV1", [P, D], F32, isOutput=False)
    b_v2 = nc.declare_dram_parameter("BV2", [P, D], F32, isOutput=False)

    out = nc.declare_dram_parameter("OT", [DT, NCH, P, CH], F32, isOutput=True)

    with tile.TileContext(nc) as tc:
        with (
            tc.tile_pool(name="w", bufs=2) as p_w,
            tc.tile_pool(name="bias", bufs=1) as p_bias,
            tc.tile_pool(name="s2", bufs=6) as p_s2,
            tc.tile_pool(name="s1", bufs=6) as p_s1,
            tc.tile_pool(name="v", bufs=2) as p_v,
            tc.tile_pool(name="esb", bufs=8) as p_esb,
            tc.tile_pool(name="small", bufs=2) as p_small,
            tc.tile_pool(name="ps", bufs=8, space="PSUM") as p_ps,
            tc.tile_pool(name="dram", bufs=1, space="DRAM") as p_dram,
        ):
            # ---- DRAM intermediates, blocked [ch, mt, p, c] ----
            e1_d = p_dram.tile([NCH, NT, P, CH], ar_dtype, tag="e1")
            e2_d = p_dram.tile([NCH, NT, P, CH], ar_dtype, tag="e2")
            z1_h = [
                p_dram.tile([2, NT, P, CH], ar_dtype, tag=f"z1{h}",
                            addr_space="Shared", name=f"z1{h}")
                for h in range(2)
            ]
            z2_h = [
                p_dram.tile([2, NT, P, CH], ar_dtype, tag=f"z2{h}",
                            addr_space="Shared", name=f"z2{h}")
                for h in range(2)
            ]

            # ---- resident loads ----
            xt_sb = p_s2.tile([P, DT, N], BF16, tag="s2", name="xt")
            yt_sb = p_s2.tile([P, DT, N], BF16, tag="s2", name="yt")
            nc.sync.dma_start(xt_sb[:], xtb[:])
            nc.sync.dma_start(yt_sb[:], ytb[:])

            bq1_sb = p_bias.tile([P, DT], F32, tag="bq1")
            bk1_sb = p_bias.tile([P, DT], F32, tag="bk1")
            bq2_sb = p_bias.tile([P, DT], F32, tag="bq2")
            bk2_sb = p_bias.tile([P, DT], F32, tag="bk2")
            bv1_sb = p_bias.tile([P, D], F32, tag="bv1")
            bv2_sb = p_bias.tile([P, D], F32, tag="bv2")
            nc.sync.dma_start(bq1_sb[:], b_q1[:])
            nc.sync.dma_start(bk1_sb[:], b_k1[:])
            nc.sync.dma_start(bq2_sb[:], b_q2[:])
            nc.sync.dma_start(bk2_sb[:], b_k2[:])
            nc.sync.dma_start(bv1_sb[:], b_v1[:])
            nc.sync.dma_start(bv2_sb[:], b_v2[:])

            def load_w(wp):
                w_sb = p_w.tile([P, DT, D], BF16, tag="w")
                nc.sync.dma_start(w_sb[:], wp[:])
                return w_sb

            def proj_T(w_sb, src_sb, bias_sb, name):
                """out[e, n] = sum_d W[e,d] src[n,d] + b[e], e-major bf16."""
                o_sb = p_s2.tile([P, DT, N], BF16, tag="s2", name=name)
                for eo in range(DT):
                    for ch in range(NCH):
                        ps = p_ps.tile([P, CH], F32, tag="ps")
                        for do in range(DT):
                            nc.tensor.matmul(
                                ps[:],
                                w_sb[:, do, eo * P : (eo + 1) * P],
                                src_sb[:, do, ch * CH : (ch + 1) * CH],
                                start=(do == 0),
                                stop=(do == DT - 1),
                            )
                        nc.scalar.activation(
                            o_sb[:, eo, ch * CH : (ch + 1) * CH],
                            ps[:],
                            mybir.ActivationFunctionType.Identity,
                            bias=bias_sb[:, eo : eo + 1],
                        )
                return o_sb

            def proj_V(w_sb, src_sb, bias_sb, name):
                """out[m, e] = sum_d src[m,d] W[e,d] + b[e], m-major bf16."""
                o_sb = p_v.tile([P, NT, D], BF16, tag="v", name=name)
                for mt in range(NT):
                    ps = p_ps.tile([P, CH], F32, tag="ps")
                    for do in range(DT):
                        nc.tensor.matmul(
                            ps[:],
                            src_sb[:, do, mt * P : (mt + 1) * P],
                            w_sb[:, do, :],
                            start=(do == 0),
                            stop=(do == DT - 1),
                        )
                    nc.vector.tensor_add(out=o_sb[:, mt, :], in0=ps[:], in1=bias_sb[:])
                return o_sb

            def scores_exp(kt_sb, qt_sb, e_dram, chs):
                """E[ch, mt, p, c] = exp(sum_e K[m,e] Q[n,e]) -> blocked DRAM."""
                for ch in chs:
                    for mt in range(NT):
                        ps = p_ps.tile([P, CH], F32, tag="ps")
                        for eo in range(DT):
                            nc.tensor.matmul(
                                ps[:],
                                kt_sb[:, eo, mt * P : (mt + 1) * P],
                                qt_sb[:, eo, ch * CH : (ch + 1) * CH],
                                start=(eo == 0),
                                stop=(eo == DT - 1),
                            )
                        e_sb = p_esb.tile([P, CH], ar_dtype, tag="esb")
                        nc.scalar.activation(
                            e_sb[:], ps[:], mybir.ActivationFunctionType.Exp
                        )
                        nc.scalar.dma_start(e_dram[ch, mt], e_sb[:])

            def ar_full(e_d, z_d):
                nc.gpsimd.collective_compute(
                    "AllReduce",
                    mybir.AluOpType.add,
                    replica_groups=[list(range(NCORES))],
                    ins=[e_d.opt()],
                    outs=[z_d.opt()],
                )

            def ar_half(e_d, z_halves, h):
                sl = slice(2 * h, 2 * h + 2)
                nc.gpsimd.collective_compute(
                    "AllReduce",
                    mybir.AluOpType.add,
                    replica_groups=[list(range(NCORES))],
                    ins=[e_d[sl].opt()],
                    outs=[z_halves[h][:].opt()],
                )

            # ======== phase A: all four Q/K projections ========
            w_sb = load_w(w_q1)
            q1t = proj_T(w_sb, xt_sb, bq1_sb, "q1t")
            w_sb = load_w(w_k2)
            k2t = proj_T(w_sb, yt_sb, bk2_sb, "k2t")
            w_sb = load_w(w_k1)
            k1t = proj_T(w_sb, xt_sb, bk1_sb, "k1t")
            w_sb = load_w(w_q2)
            q2t = proj_T(w_sb, yt_sb, bq2_sb, "q2t")

            # ======== phase B: interleaved halves, alternating ARs ========
            scores_exp(k2t, q1t, e1_d, (0, 1))
            ar_half(e1_d, z1_h, 0)
            scores_exp(k1t, q2t, e2_d, (0, 1))
            ar_half(e2_d, z2_h, 0)
            scores_exp(k2t, q1t, e1_d, (2, 3))
            ar_half(e1_d, z1_h, 1)
            scores_exp(k1t, q2t, e2_d, (2, 3))
            ar_half(e2_d, z2_h, 1)

            # ======== phase A3 (V projections; fills the AR window) ========
            w_sb = load_w(w_v2)
            v2 = proj_V(w_sb, yt_sb, bv2_sb, "v2")
            w_sb = load_w(w_v1)
            v1 = proj_V(w_sb, xt_sb, bv1_sb, "v1")

            # ======== phase C ========
            # Split into a U1 pass (needs only Z1, runs under AllReduce #2)
            # and a U2 pass (after Z2). U1 partial sums are HELD OPEN in
            # PSUM banks (4 per chunk); the U2 matmuls append into the same
            # accumulation group. Emission order hand-interleaves chunks so
            # the in-order PE stream never parks behind a not-yet-ready
            # chunk while a ready one waits.
            def make_a_half(e_d, z_src, ch, h, name, mult_eng=None):
                """A[:, 8 mt, c] = E/Z for chunk ch, half h; bf16 [P, HT, CH]."""
                msl = slice(h * HT, (h + 1) * HT)
                e_b = p_s2.tile([P, HT, CH], ar_dtype, tag="s2", name=f"eb{name}")
                nc.sync.dma_start(e_b[:], e_d[ch, msl].rearrange("m p c -> p m c"))
                z_b = p_s2.tile([P, HT, CH], ar_dtype, tag="s2", name=f"zb{name}")
                nc.sync.dma_start(z_b[:], z_src(ch, msl))
                z_f = p_s2.tile([P, HT, CH], F32, tag="s2", name=f"zf{name}")
                nc.scalar.activation(
                    z_f[:], z_b[:], mybir.ActivationFunctionType.Copy
                )
                rz = p_s2.tile([P, HT, CH], F32, tag="s2", name=f"rz{name}")
                nc.vector.reciprocal_approx_fast(out=rz[:], in_=z_f[:])
                a_sb = p_s1.tile([P, HT, CH], BF16, tag="s1", name=f"a{name}")
                (mult_eng or nc.vector).tensor_mul(
                    out=a_sb[:], in0=e_b[:], in1=rz[:]
                )
                return a_sb

            def z1_src(ch, msl):
                return z1_h[ch // 2][ch % 2, msl].rearrange("m p c -> p m c")

            def z2_src(ch, msl):
                return z2_h[ch // 2][ch % 2, msl].rearrange("m p c -> p m c")

            ps_held = {}

            def u1_pass(ch):
                a1 = [make_a_half(e1_d, z1_src, ch, h, f"1{h}") for h in range(2)]
                tiles = []
                for dt in range(DT):
                    dsl = slice(dt * P, (dt + 1) * P)
                    ps = p_ps.tile([P, CH], F32, tag="ps")
                    for mt in range(NT):
                        nc.tensor.matmul(
                            ps[:], v2[:, mt, dsl], a1[mt // HT][:, mt % HT, :],
                            start=(mt == 0), stop=False,
                        )
                    tiles.append(ps)
                ps_held[ch] = tiles

            def u2_pass(ch):
                a2 = [make_a_half(e2_d, z2_src, ch, h, f"2{h}") for h in range(2)]
                for dt in range(DT):
                    dsl = slice(dt * P, (dt + 1) * P)
                    ps = ps_held[ch][dt]
                    for mt in range(NT):
                        nc.tensor.matmul(
                            ps[:], v1[:, mt, dsl], a2[mt // HT][:, mt % HT, :],
                            start=False, stop=(mt == NT - 1),
                        )
                    xyres = p_small.tile([P, CH], F32, tag="xyres")
                    nc.scalar.dma_start(xyres[:], xyf[dt, ch])
                    ot = p_small.tile([P, CH], F32, tag="ot")
                    nc.vector.tensor_add(out=ot[:], in0=ps[:], in1=xyres[:])
                    nc.scalar.dma_start(out[dt, ch], ot[:])

            u1_pass(0)
            u1_pass(1)
            u2_pass(0)
            u2_pass(1)
            u1_pass(2)
            u1_pass(3)
            u2_pass(2)
            u2_pass(3)

    nc.compile()
    return nc


def _pmajor(a, inner):
    """[O*P, F] -> [P, O, F] partition-major."""
    o = a.shape[0] // inner
    return np.ascontiguousarray(a.reshape(o, inner, a.shape[1]).transpose(1, 0, 2))


def _blocked(a):
    """[D, N] -> [DT, NCH, P, CH] blocked."""
    return np.ascontiguousarray(a.reshape(DT, P, NCH, CH).transpose(0, 2, 1, 3))


def _prep_inputs(inputs):
    import ml_dtypes

    X = np.asarray(inputs["X"], dtype=np.float32)
    Y = np.asarray(inputs["Y"], dtype=np.float32)
    scale = np.float32(1.0 / np.sqrt(D))

    def wT(name, s=np.float32(1.0)):
        w = np.asarray(inputs[f"W_{name}"], dtype=np.float32)
        return _pmajor((w.T * s).astype(ml_dtypes.bfloat16), P)

    def bstripe(name, s=np.float32(1.0)):
        b = np.asarray(inputs[f"b_{name}"], dtype=np.float32) * s
        return np.ascontiguousarray(b.reshape(DT, P).T)

    def bbcast(name):
        b = np.asarray(inputs[f"b_{name}"], dtype=np.float32)
        return np.ascontiguousarray(np.broadcast_to(b, (P, D)))

    shared = {
        "WQ1T": wT("xq", scale),
        "WK1T": wT("xk"),
        "WV1T": wT("xv"),
        "WQ2T": wT("yq", scale),
        "WK2T": wT("yk"),
        "WV2T": wT("yv"),
        "BQ1": bstripe("xq", scale),
        "BK1": bstripe("xk"),
        "BQ2": bstripe("yq", scale),
        "BK2": bstripe("yk"),
        "BV1": bbcast("xv"),
        "BV2": bbcast("yv"),
    }
    in_maps = []
    for c in range(NCORES):
        xt = np.ascontiguousarray(X[c].T)
        yt = np.ascontiguousarray(Y[c].T)
        m = dict(shared)
        m["XYF"] = _blocked(xt + yt)
        m["XTB"] = _pmajor(xt.astype(ml_dtypes.bfloat16), P)
        m["YTB"] = _pmajor(yt.astype(ml_dtypes.bfloat16), P)
        in_maps.append(m)
    return in_maps


def _unblock(ot):
    """[DT, NCH, P, CH] -> [N, D] (transposed back)."""
    return ot.transpose(0, 2, 1, 3).reshape(D, N).T


def kernel(**inputs):
    if "nc" not in _CACHE:
        _CACHE["nc"] = build()
    nc = _CACHE["nc"]
    in_maps = _prep_inputs(inputs)
    res = run_bass_kernel_spmd(
        nc, in_maps, core_ids=list(range(NCORES)), **_CACHE.get("run_kwargs", {})
    )
    _CACHE["last_result"] = res
    out = np.stack(
        [np.ascontiguousarray(_unblock(res.results[c]["OT"])) for c in range(NCORES)]
    )
    return out.astype(np.float32)


# revision 28
# speedup vs baseline: 1.0722x; 1.0139x over previous
"""Trainium2 Bass kernel for nn_CA_82300163326040.

Cross-attention between X and Y with softmax over the BATCH axis (torch
legacy dim=0). B=8, N=2048, D=512, f32.

Sharding: data-parallel over batch, one batch per NeuronCore (8 cores).
The batch-axis softmax couples cores: each core computes its local
exp-scores E1=exp(Q1.K2^T*s), E2=exp(Q2.K1^T*s) ([2048,2048]) and the
denominators Z = sum_b E are obtained with 8-core bf16 AllReduces.
No max-subtraction is needed: S ~ N(0, 1/9) so exp() cannot overflow.

Compute dtype: bf16 matmuls (PE full rate), f32 PSUM + softmax plumbing.
All DRAM layouts are pre-blocked on host so every device DMA moves large
contiguous chunks; E/Z are blocked [ch, mt, p, 512] (n-chunk major) so
phase-B writes one 128KB block per score tile and phase C stages whole
2MB chunks contiguously.

Per-core phases:
  A: six projections. Q/K transposed [e,n] via ACT-bias eviction; V
     row-major [m,e] via DVE bias add. 1/sqrt(D) folded into Q weights.
  B: T[m,n] = S^T tiles via bf16 matmuls, exp fused into PSUM eviction
     (ACT), E -> blocked DRAM on the gpsimd queue. AllReduce E -> Z.
  C: per n-chunk: stage Z/E halves, ACT-copy Z to f32, DVE fast-recip,
     A = E * rZ (mixed-dtype DVE mult -> bf16); U^T[d,n] accumulates
     V2^T A1 + V1^T A2 in PSUM (32 matmuls per [128,512] tile),
     eviction adds the (X+Y)^T residual. Output blocked, host unblocks.
"""

import numpy as np

import concourse.bass as bass
import concourse.mybir as mybir
import concourse.tile as tile
from concourse import bacc
from concourse.bass_utils import run_bass_kernel_spmd

P = 128
N = 2048  # sequence length
D = 512  # model dim
NCORES = 8
DT = D // P  # 4 feature tiles
NT = N // P  # 16 sequence tiles
CH = 512  # n-chunk (free dim of all matmuls)
NCH = N // CH  # 4 chunks
HT = NT // 2  # half of the m-tiles (staging granularity)

F32 = mybir.dt.float32
BF16 = mybir.dt.bfloat16
F8 = mybir.dt.float8e4

_CACHE = {}


def build(ar_dtype=BF16):
    nc = bacc.Bacc("TRN2", target_bir_lowering=False, debug=False, num_devices=NCORES)

    # ---- parameters (per core), all pre-arranged on host ----
    xtb = nc.declare_dram_parameter("XTB", [P, DT, N], BF16, isOutput=False)
    ytb = nc.declare_dram_parameter("YTB", [P, DT, N], BF16, isOutput=False)
    # combined residual (X+Y)^T, blocked [dt, ch, p, c], f32
    xyf = nc.declare_dram_parameter("XYF", [DT, NCH, P, CH], F32, isOutput=False)
    # weights (transposed, partition-major): w[p, o, e] = W^T[o*128+p, e]
    w_q1 = nc.declare_dram_parameter("WQ1T", [P, DT, D], BF16, isOutput=False)
    w_k1 = nc.declare_dram_parameter("WK1T", [P, DT, D], BF16, isOutput=False)
    w_v1 = nc.declare_dram_parameter("WV1T", [P, DT, D], BF16, isOutput=False)
    w_q2 = nc.declare_dram_parameter("WQ2T", [P, DT, D], BF16, isOutput=False)
    w_k2 = nc.declare_dram_parameter("WK2T", [P, DT, D], BF16, isOutput=False)
    w_v2 = nc.declare_dram_parameter("WV2T", [P, DT, D], BF16, isOutput=False)
    b_q1 = nc.declare_dram_parameter("BQ1", [P, DT], F32, isOutput=False)
    b_k1 = nc.declare_dram_parameter("BK1", [P, DT], F32, isOutput=False)
    b_q2 = nc.declare_dram_parameter("BQ2", [P, DT], F32, isOutput=False)
    b_k2 = nc.declare_dram_parameter("BK2", [P, DT], F32, isOutput=False)
    b_v1 = nc.declare_dram_parameter("BV1", [P, D], F32, isOutput=False)
    b_v2 = nc.declare_dram_parameter("BV2", [P, D], F32, isOutput=False)

    out = nc.declare_dram_parameter("OT", [DT, NCH, P, CH], F32, isOutput=True)

    with tile.TileContext(nc) as tc:
        with (
            tc.tile_pool(name="w", bufs=2) as p_w,
            tc.tile_pool(name="bias", bufs=1) as p_bias,
            tc.tile_pool(name="s2", bufs=6) as p_s2,
            tc.tile_pool(name="s1", bufs=6) as p_s1,
            tc.tile_pool(name="v", bufs=2) as p_v,
            tc.tile_pool(name="esb", bufs=8) as p_esb,
            tc.tile_pool(name="small", bufs=2) as p_small,
            tc.tile_pool(name="ps", bufs=8, space="PSUM") as p_ps,
            tc.tile_pool(name="dram", bufs=1, space="DRAM") as p_dram,
        ):
            # ---- DRAM intermediates, blocked [ch, mt, p, c] ----
            e1_d = p_dram.tile([NCH, NT, P, CH], ar_dtype, tag="e1")
            e2_d = p_dram.tile([NCH, NT, P, CH], ar_dtype, tag="e2")
            z1_h = [
                p_dram.tile([2, NT, P, CH], ar_dtype, tag=f"z1{h}",
                            addr_space="Shared", name=f"z1{h}")
                for h in range(2)
            ]
            z2_h = [
                p_dram.tile([2, NT, P, CH], ar_dtype, tag=f"z2{h}",
                            addr_space="Shared", name=f"z2{h}")
                for h in range(2)
            ]

            # ---- resident loads ----
            xt_sb = p_s2.tile([P, DT, N], BF16, tag="s2", name="xt")
            yt_sb = p_s2.tile([P, DT, N], BF16, tag="s2", name="yt")
            nc.sync.dma_start(xt_sb[:], xtb[:])
            nc.sync.dma_start(yt_sb[:], ytb[:])

            bq1_sb = p_bias.tile([P, DT], F32, tag="bq1")
            bk1_sb = p_bias.tile([P, DT], F32, tag="bk1")
            bq2_sb = p_bias.tile([P, DT], F32, tag="bq2")
            bk2_sb = p_bias.tile([P, DT], F32, tag="bk2")
            bv1_sb = p_bias.tile([P, D], F32, tag="bv1")
            bv2_sb = p_bias.tile([P, D], F32, tag="bv2")
            nc.sync.dma_start(bq1_sb[:], b_q1[:])
            nc.sync.dma_start(bk1_sb[:], b_k1[:])
            nc.sync.dma_start(bq2_sb[:], b_q2[:])
            nc.sync.dma_start(bk2_sb[:], b_k2[:])
            nc.sync.dma_start(bv1_sb[:], b_v1[:])
            nc.sync.dma_start(bv2_sb[:], b_v2[:])

            def load_w(wp):
                w_sb = p_w.tile([P, DT, D], BF16, tag="w")
                nc.sync.dma_start(w_sb[:], wp[:])
                return w_sb

            def proj_T(w_sb, src_sb, bias_sb, name):
                """out[e, n] = sum_d W[e,d] src[n,d] + b[e], e-major bf16."""
                o_sb = p_s2.tile([P, DT, N], BF16, tag="s2", name=name)
                for eo in range(DT):
                    for ch in range(NCH):
                        ps = p_ps.tile([P, CH], F32, tag="ps")
                        for do in range(DT):
                            nc.tensor.matmul(
                                ps[:],
                                w_sb[:, do, eo * P : (eo + 1) * P],
                                src_sb[:, do, ch * CH : (ch + 1) * CH],
                                start=(do == 0),
                                stop=(do == DT - 1),
                            )
                        nc.scalar.activation(
                            o_sb[:, eo, ch * CH : (ch + 1) * CH],
                            ps[:],
                            mybir.ActivationFunctionType.Identity,
                            bias=bias_sb[:, eo : eo + 1],
                        )
                return o_sb

            def proj_V(w_sb, src_sb, bias_sb, name):
                """out[m, e] = sum_d src[m,d] W[e,d] + b[e], m-major bf16."""
                o_sb = p_v.tile([P, NT, D], BF16, tag="v", name=name)
                for mt in range(NT):
                    ps = p_ps.tile([P, CH], F32, tag="ps")
                    for do in range(DT):
                        nc.tensor.matmul(
                            ps[:],
                            src_sb[:, do, mt * P : (mt + 1) * P],
                            w_sb[:, do, :],
                            start=(do == 0),
                            stop=(do == DT - 1),
                        )
                    nc.vector.tensor_add(out=o_sb[:, mt, :], in0=ps[:], in1=bias_sb[:])
                return o_sb

            def scores_exp(kt_sb, qt_sb, e_dram, chs):
                """E[ch, mt, p, c] = exp(sum_e K[m,e] Q[n,e]) -> blocked DRAM."""
                for ch in chs:
                    for mt in range(NT):
                        ps = p_ps.tile([P, CH], F32, tag="ps")
                        for eo in range(DT):
                            nc.tensor.matmul(
                                ps[:],
                                kt_sb[:, eo, mt * P : (mt + 1) * P],
                                qt_sb[:, eo, ch * CH : (ch + 1) * CH],
                                start=(eo == 0),
                                stop=(eo == DT - 1),
                            )
                        e_sb = p_esb.tile([P, CH], ar_dtype, tag="esb")
                        nc.scalar.activation(
                            e_sb[:], ps[:], mybir.ActivationFunctionType.Exp
                        )
                        nc.scalar.dma_start(e_dram[ch, mt], e_sb[:])

            def ar_full(e_d, z_d):
                nc.gpsimd.collective_compute(
                    "AllReduce",
                    mybir.AluOpType.add,
                    replica_groups=[list(range(NCORES))],
                    ins=[e_d.opt()],
                    outs=[z_d.opt()],
                )

            def ar_half(e_d, z_halves, h):
                sl = slice(2 * h, 2 * h + 2)
                nc.gpsimd.collective_compute(
                    "AllReduce",
                    mybir.AluOpType.add,
                    replica_groups=[list(range(NCORES))],
                    ins=[e_d[sl].opt()],
                    outs=[z_halves[h][:].opt()],
                )

            # ======== phase A: all four Q/K projections ========
            w_sb = load_w(w_q1)
            q1t = proj_T(w_sb, xt_sb, bq1_sb, "q1t")
            w_sb = load_w(w_k2)
            k2t = proj_T(w_sb, yt_sb, bk2_sb, "k2t")
            w_sb = load_w(w_k1)
            k1t = proj_T(w_sb, xt_sb, bk1_sb, "k1t")
            w_sb = load_w(w_q2)
            q2t = proj_T(w_sb, yt_sb, bq2_sb, "q2t")

            # ======== phase B: interleaved halves, alternating ARs ========
            scores_exp(k2t, q1t, e1_d, (0, 1))
            ar_half(e1_d, z1_h, 0)
            scores_exp(k1t, q2t, e2_d, (0, 1))
            ar_half(e2_d, z2_h, 0)
            scores_exp(k2t, q1t, e1_d, (2, 3))
            ar_half(e1_d, z1_h, 1)
            scores_exp(k1t, q2t, e2_d, (2, 3))
            ar_half(e2_d, z2_h, 1)

            # ======== phase A3 (V projections; fills the AR window) ========
            w_sb = load_w(w_v2)
            v2 = proj_V(w_sb, yt_sb, bv2_sb, "v2")
            w_sb = load_w(w_v1)
            v1 = proj_V(w_sb, xt_sb, bv1_sb, "v1")

            # ======== phase C ========
            # Split into a U1 pass (needs only Z1, runs under AllReduce #2)
            # and a U2 pass (after Z2). U1 partial sums are HELD OPEN in
            # PSUM banks (4 per chunk); the U2 matmuls append into the same
            # accumulation group. Emission order hand-interleaves chunks so
            # the in-order PE stream never parks behind a not-yet-ready
            # chunk while a ready one waits.
            def make_a_half(e_d, z_src, ch, h, name, mult_eng=None):
                """A[:, 8 mt, c] = E/Z for chunk ch, half h; bf16 [P, HT, CH]."""
                msl = slice(h * HT, (h + 1) * HT)
                z_b = p_s1.tile([P, HT, CH], ar_dtype, tag="s1", name=f"zb{name}")
                nc.sync.dma_start(z_b[:], z_src(ch, msl))
                z_f = p_s2.tile([P, HT, CH], F32, tag="s2", name=f"zf{name}")
                nc.scalar.activation(
                    z_f[:], z_b[:], mybir.ActivationFunctionType.Copy
                )
                rz = p_s2.tile([P, HT, CH], F32, tag="s2", name=f"rz{name}")
                nc.vector.reciprocal_approx_fast(out=rz[:], in_=z_f[:])
                e_b = p_s1.tile([P, HT, CH], ar_dtype, tag="s1", name=f"eb{name}")
                nc.sync.dma_start(e_b[:], e_d[ch, msl].rearrange("m p c -> p m c"))
                a_sb = p_s1.tile([P, HT, CH], BF16, tag="s1", name=f"a{name}")
                (mult_eng or nc.vector).tensor_mul(
                    out=a_sb[:], in0=e_b[:], in1=rz[:]
                )
                return a_sb

            def z1_src(ch, msl):
                return z1_h[ch // 2][ch % 2, msl].rearrange("m p c -> p m c")

            def z2_src(ch, msl):
                return z2_h[ch // 2][ch % 2, msl].rearrange("m p c -> p m c")

            ps_held = {}

            def u1_pass(ch):
                a1 = [make_a_half(e1_d, z1_src, ch, h, f"1{h}") for h in range(2)]
                tiles = []
                for dt in range(DT):
                    dsl = slice(dt * P, (dt + 1) * P)
                    ps = p_ps.tile([P, CH], F32, tag="ps")
                    for mt in range(NT):
                        nc.tensor.matmul(
                            ps[:], v2[:, mt, dsl], a1[mt // HT][:, mt % HT, :],
                            start=(mt == 0), stop=False,
                        )
                    tiles.append(ps)
                ps_held[ch] = tiles

            def u2_pass(ch):
                a2 = [make_a_half(e2_d, z2_src, ch, h, f"2{h}") for h in range(2)]
                for dt in range(DT):
                    dsl = slice(dt * P, (dt + 1) * P)
                    ps = ps_held[ch][dt]
                    for mt in range(NT):
                        nc.tensor.matmul(
                            ps[:], v1[:, mt, dsl], a2[mt // HT][:, mt % HT, :],
                            start=False, stop=(mt == NT - 1),
                        )
                    xyres = p_small.tile([P, CH], F32, tag="xyres")
                    nc.scalar.dma_start(xyres[:], xyf[dt, ch])
                    ot = p_small.tile([P, CH], F32, tag="ot")
                    nc.vector.tensor_add(out=ot[:], in0=ps[:], in1=xyres[:])
                    nc.scalar.dma_start(out[dt, ch], ot[:])

            u1_pass(0)
            u1_pass(1)
            u2_pass(0)
            u2_pass(1)
            u1_pass(2)
            u1_pass(3)
            u2_pass(2)
            u2_pass(3)

    nc.compile()
    return nc


def _pmajor(a, inner):
    """[O*P, F] -> [P, O, F] partition-major."""
    o = a.shape[0] // inner
    return np.ascontiguousarray(a.reshape(o, inner, a.shape[1]).transpose(1, 0, 2))


def _blocked(a):
    """[D, N] -> [DT, NCH, P, CH] blocked."""
    return np.ascontiguousarray(a.reshape(DT, P, NCH, CH).transpose(0, 2, 1, 3))


def _prep_inputs(inputs):
    import ml_dtypes

    X = np.asarray(inputs["X"], dtype=np.float32)
    Y = np.asarray(inputs["Y"], dtype=np.float32)
    scale = np.float32(1.0 / np.sqrt(D))

    def wT(name, s=np.float32(1.0)):
        w = np.asarray(inputs[f"W_{name}"], dtype=np.float32)
        return _pmajor((w.T * s).astype(ml_dtypes.bfloat16), P)

    def bstripe(name, s=np.float32(1.0)):
        b = np.asarray(inputs[f"b_{name}"], dtype=np.float32) * s
        return np.ascontiguousarray(b.reshape(DT, P).T)

    def bbcast(name):
        b = np.asarray(inputs[f"b_{name}"], dtype=np.float32)
        return np.ascontiguousarray(np.broadcast_to(b, (P, D)))

    shared = {
        "WQ1T": wT("xq", scale),
        "WK1T": wT("xk"),
        "WV1T": wT("xv"),
        "WQ2T": wT("yq", scale),
        "WK2T": wT("yk"),
        "WV2T": wT("yv"),
        "BQ1": bstripe("xq", scale),
        "BK1": bstripe("xk"),
        "BQ2": bstripe("yq", scale),
        "BK2": bstripe("yk"),
        "BV1": bbcast("xv"),
        "BV2": bbcast("yv"),
    }
    in_maps = []
    for c in range(NCORES):
        xt = np.ascontiguousarray(X[c].T)
        yt = np.ascontiguousarray(Y[c].T)
        m = dict(shared)
        m["XYF"] = _blocked(xt + yt)
        m["XTB"] = _pmajor(xt.astype(ml_dtypes.bfloat16), P)
        m["YTB"] = _pmajor(yt.astype(ml_dtypes.bfloat16), P)
        in_maps.append(m)
    return in_maps


def _unblock(ot):
    """[DT, NCH, P, CH] -> [N, D] (transposed back)."""
    return ot.transpose(0, 2, 1, 3).reshape(D, N).T


def kernel(**inputs):
    if "nc" not in _CACHE:
        _CACHE["nc"] = build()
    nc = _CACHE["nc"]
    in_maps = _prep_inputs(inputs)
    res = run_bass_kernel_spmd(
        nc, in_maps, core_ids=list(range(NCORES)), **_CACHE.get("run_kwargs", {})
    )
    _CACHE["last_result"] = res
    out = np.stack(
        [np.ascontiguousarray(_unblock(res.results[c]["OT"])) for c in range(NCORES)]
    )
    return out.astype(np.float32)


# revision 33
# speedup vs baseline: 1.0921x; 1.0186x over previous
"""Trainium2 Bass kernel for nn_CA_82300163326040.

Cross-attention between X and Y with softmax over the BATCH axis (torch
legacy dim=0). B=8, N=2048, D=512, f32.

Sharding: data-parallel over batch, one batch per NeuronCore (8 cores).
The batch-axis softmax couples cores: each core computes its local
exp-scores E1=exp(Q1.K2^T*s), E2=exp(Q2.K1^T*s) ([2048,2048]) and the
denominators Z = sum_b E are obtained with 8-core bf16 AllReduces.
No max-subtraction is needed: S ~ N(0, 1/9) so exp() cannot overflow.

Compute dtype: bf16 matmuls (PE full rate), f32 PSUM + softmax plumbing.
All DRAM layouts are pre-blocked on host so every device DMA moves large
contiguous chunks; E/Z are blocked [ch, mt, p, 512] (n-chunk major) so
phase-B writes one 128KB block per score tile and phase C stages whole
chunks contiguously.

Per-core schedule (chosen from ~10 profiled variants):
  A: four Q/K projections (transposed [e,n], bias fused in the ACT
     PSUM-eviction; 1/sqrt(D) folded into Q weights on host).
  B: score tiles T=S^T via bf16 matmuls, exp fused into the PSUM
     eviction (ACT), E -> blocked DRAM. B1/B2 halves interleaved with
     four quarter-AllReduces alternating E1/E2 so the collective engine
     runs back-to-back and Z2 chunks arrive early.
  A3: V projections (row-major [m,e], DVE bias add) fill the AR window.
  C: split U1/U2 passes. U1 = V2^T A1 accumulates into PSUM banks that
     are HELD OPEN; U2 = V1^T A2 appends into the same accumulation
     groups once its Z chunk lands, then evicts adding the (X+Y)^T
     residual. A = E * recip(Z): Z staged bf16 -> ACT copy to f32 ->
     DVE reciprocal_approx_fast -> mixed-dtype DVE multiply -> bf16.
     Chunk order is hand-interleaved so the in-order PE stream never
     parks behind a not-yet-ready chunk. Output blocked; host unblocks.
"""

import numpy as np

import concourse.bass as bass
import concourse.mybir as mybir
import concourse.tile as tile
from concourse import bacc
from concourse.bass_utils import run_bass_kernel_spmd

P = 128
N = 2048  # sequence length
D = 512  # model dim
NCORES = 8
DT = D // P  # 4 feature tiles
NT = N // P  # 16 sequence tiles
CH = 512  # n-chunk (free dim of all matmuls)
NCH = N // CH  # 4 chunks
HT = NT // 2  # half of the m-tiles (staging granularity)

F32 = mybir.dt.float32
BF16 = mybir.dt.bfloat16
F8 = mybir.dt.float8e4

_CACHE = {}


def build(ar_dtype=BF16):
    nc = bacc.Bacc("TRN2", target_bir_lowering=False, debug=False, num_devices=NCORES)

    # ---- parameters (per core), all pre-arranged on host ----
    xtb = nc.declare_dram_parameter("XTB", [P, DT, N], BF16, isOutput=False)
    ytb = nc.declare_dram_parameter("YTB", [P, DT, N], BF16, isOutput=False)
    # combined residual (X+Y)^T, blocked [dt, ch, p, c], f32
    xyf = nc.declare_dram_parameter("XYF", [DT, NCH, P, CH], F32, isOutput=False)
    # weights (transposed, partition-major): w[p, o, e] = W^T[o*128+p, e]
    w_q1 = nc.declare_dram_parameter("WQ1T", [P, DT, D], BF16, isOutput=False)
    w_k1 = nc.declare_dram_parameter("WK1T", [P, DT, D], BF16, isOutput=False)
    w_v1 = nc.declare_dram_parameter("WV1T", [P, DT, D], BF16, isOutput=False)
    w_q2 = nc.declare_dram_parameter("WQ2T", [P, DT, D], BF16, isOutput=False)
    w_k2 = nc.declare_dram_parameter("WK2T", [P, DT, D], BF16, isOutput=False)
    w_v2 = nc.declare_dram_parameter("WV2T", [P, DT, D], BF16, isOutput=False)
    b_q1 = nc.declare_dram_parameter("BQ1", [P, DT], F32, isOutput=False)
    b_k1 = nc.declare_dram_parameter("BK1", [P, DT], F32, isOutput=False)
    b_q2 = nc.declare_dram_parameter("BQ2", [P, DT], F32, isOutput=False)
    b_k2 = nc.declare_dram_parameter("BK2", [P, DT], F32, isOutput=False)
    b_v1 = nc.declare_dram_parameter("BV1", [P, D], F32, isOutput=False)
    b_v2 = nc.declare_dram_parameter("BV2", [P, D], F32, isOutput=False)

    out = nc.declare_dram_parameter("OT", [DT, NCH, P, CH], F32, isOutput=True)

    with tile.TileContext(nc) as tc:
        with (
            tc.tile_pool(name="w", bufs=2) as p_w,
            tc.tile_pool(name="bias", bufs=1) as p_bias,
            tc.tile_pool(name="s2", bufs=6) as p_s2,
            tc.tile_pool(name="s1", bufs=6) as p_s1,
            tc.tile_pool(name="v", bufs=2) as p_v,
            tc.tile_pool(name="esb", bufs=8) as p_esb,
            tc.tile_pool(name="small", bufs=2) as p_small,
            tc.tile_pool(name="ps", bufs=8, space="PSUM") as p_ps,
            tc.tile_pool(name="dram", bufs=1, space="DRAM") as p_dram,
        ):
            # ---- DRAM intermediates, blocked [ch, mt, p, c] ----
            e1_d = p_dram.tile([NCH, NT, P, CH], ar_dtype, tag="e1")
            e2_d = p_dram.tile([NCH, NT, P, CH], ar_dtype, tag="e2")
            z1_h = [
                p_dram.tile([2, NT, P, CH], ar_dtype, tag=f"z1{h}",
                            addr_space="Shared", name=f"z1{h}")
                for h in range(2)
            ]
            z2_h = [
                p_dram.tile([2, NT, P, CH], ar_dtype, tag=f"z2{h}",
                            addr_space="Shared", name=f"z2{h}")
                for h in range(2)
            ]

            # ---- resident loads ----
            xt_sb = p_s2.tile([P, DT, N], BF16, tag="s2", name="xt")
            yt_sb = p_s2.tile([P, DT, N], BF16, tag="s2", name="yt")
            nc.sync.dma_start(xt_sb[:], xtb[:])
            nc.sync.dma_start(yt_sb[:], ytb[:])

            bq1_sb = p_bias.tile([P, DT], F32, tag="bq1")
            bk1_sb = p_bias.tile([P, DT], F32, tag="bk1")
            bq2_sb = p_bias.tile([P, DT], F32, tag="bq2")
            bk2_sb = p_bias.tile([P, DT], F32, tag="bk2")
            bv1_sb = p_bias.tile([P, D], F32, tag="bv1")
            bv2_sb = p_bias.tile([P, D], F32, tag="bv2")
            nc.sync.dma_start(bq1_sb[:], b_q1[:])
            nc.sync.dma_start(bk1_sb[:], b_k1[:])
            nc.sync.dma_start(bq2_sb[:], b_q2[:])
            nc.sync.dma_start(bk2_sb[:], b_k2[:])
            nc.sync.dma_start(bv1_sb[:], b_v1[:])
            nc.sync.dma_start(bv2_sb[:], b_v2[:])

            def load_w(wp):
                w_sb = p_w.tile([P, DT, D], BF16, tag="w")
                nc.sync.dma_start(w_sb[:], wp[:])
                return w_sb

            def proj_T(w_sb, src_sb, bias_sb, name):
                """out[e, n] = sum_d W[e,d] src[n,d] + b[e], e-major bf16."""
                o_sb = p_s2.tile([P, DT, N], BF16, tag="s2", name=name)
                for eo in range(DT):
                    for ch in range(NCH):
                        ps = p_ps.tile([P, CH], F32, tag="ps")
                        for do in range(DT):
                            nc.tensor.matmul(
                                ps[:],
                                w_sb[:, do, eo * P : (eo + 1) * P],
                                src_sb[:, do, ch * CH : (ch + 1) * CH],
                                start=(do == 0),
                                stop=(do == DT - 1),
                            )
                        nc.scalar.activation(
                            o_sb[:, eo, ch * CH : (ch + 1) * CH],
                            ps[:],
                            mybir.ActivationFunctionType.Identity,
                            bias=bias_sb[:, eo : eo + 1],
                        )
                return o_sb

            def proj_V(w_sb, src_sb, bias_sb, name):
                """out[m, e] = sum_d src[m,d] W[e,d] + b[e], m-major bf16."""
                o_sb = p_v.tile([P, NT, D], BF16, tag="v", name=name)
                for mt in range(NT):
                    ps = p_ps.tile([P, CH], F32, tag="ps")
                    for do in range(DT):
                        nc.tensor.matmul(
                            ps[:],
                            src_sb[:, do, mt * P : (mt + 1) * P],
                            w_sb[:, do, :],
                            start=(do == 0),
                            stop=(do == DT - 1),
                        )
                    nc.vector.tensor_add(out=o_sb[:, mt, :], in0=ps[:], in1=bias_sb[:])
                return o_sb

            def scores_exp(kt_sb, qt_sb, e_dram, chs):
                """E[ch, mt, p, c] = exp(sum_e K[m,e] Q[n,e]) -> blocked DRAM."""
                for ch in chs:
                    for mt in range(NT):
                        ps = p_ps.tile([P, CH], F32, tag="ps")
                        for eo in range(DT):
                            nc.tensor.matmul(
                                ps[:],
                                kt_sb[:, eo, mt * P : (mt + 1) * P],
                                qt_sb[:, eo, ch * CH : (ch + 1) * CH],
                                start=(eo == 0),
                                stop=(eo == DT - 1),
                            )
                        e_sb = p_esb.tile([P, CH], ar_dtype, tag="esb")
                        nc.scalar.activation(
                            e_sb[:], ps[:], mybir.ActivationFunctionType.Exp
                        )
                        nc.scalar.dma_start(e_dram[ch, mt], e_sb[:])

            def ar_full(e_d, z_d):
                nc.gpsimd.collective_compute(
                    "AllReduce",
                    mybir.AluOpType.add,
                    replica_groups=[list(range(NCORES))],
                    ins=[e_d.opt()],
                    outs=[z_d.opt()],
                )

            def ar_half(e_d, z_halves, h):
                sl = slice(2 * h, 2 * h + 2)
                nc.gpsimd.collective_compute(
                    "AllReduce",
                    mybir.AluOpType.add,
                    replica_groups=[list(range(NCORES))],
                    ins=[e_d[sl].opt()],
                    outs=[z_halves[h][:].opt()],
                )

            # ======== phase A: all four Q/K projections ========
            w_sb = load_w(w_q1)
            q1t = proj_T(w_sb, xt_sb, bq1_sb, "q1t")
            w_sb = load_w(w_k2)
            k2t = proj_T(w_sb, yt_sb, bk2_sb, "k2t")
            w_sb = load_w(w_k1)
            k1t = proj_T(w_sb, xt_sb, bk1_sb, "k1t")
            w_sb = load_w(w_q2)
            q2t = proj_T(w_sb, yt_sb, bq2_sb, "q2t")

            # ======== phase B: interleaved halves, alternating ARs ========
            scores_exp(k2t, q1t, e1_d, (0, 1))
            ar_half(e1_d, z1_h, 0)
            scores_exp(k1t, q2t, e2_d, (0, 1))
            ar_half(e2_d, z2_h, 0)
            scores_exp(k2t, q1t, e1_d, (2, 3))
            ar_half(e1_d, z1_h, 1)
            scores_exp(k1t, q2t, e2_d, (2, 3))
            ar_half(e2_d, z2_h, 1)

            # ======== phase A3 (V projections; fills the AR window) ========
            w_sb = load_w(w_v2)
            v2 = proj_V(w_sb, yt_sb, bv2_sb, "v2")
            w_sb = load_w(w_v1)
            v1 = proj_V(w_sb, xt_sb, bv1_sb, "v1")

            # ======== phase C ========
            # Split into a U1 pass (needs only Z1, runs under AllReduce #2)
            # and a U2 pass (after Z2). U1 partial sums are HELD OPEN in
            # PSUM banks (4 per chunk); the U2 matmuls append into the same
            # accumulation group. Emission order hand-interleaves chunks so
            # the in-order PE stream never parks behind a not-yet-ready
            # chunk while a ready one waits.
            def make_a_half(e_d, z_src, ch, h, name, mult_eng=None):
                """A[:, 8 mt, c] = E/Z for chunk ch, half h; bf16 [P, HT, CH]."""
                msl = slice(h * HT, (h + 1) * HT)
                z_b = p_s1.tile([P, HT, CH], ar_dtype, tag="s1", name=f"zb{name}")
                nc.sync.dma_start(z_b[:], z_src(ch, msl))
                z_f = p_s2.tile([P, HT, CH], F32, tag="s2", name=f"zf{name}")
                nc.scalar.activation(
                    z_f[:], z_b[:], mybir.ActivationFunctionType.Copy
                )
                rz = p_s2.tile([P, HT, CH], F32, tag="s2", name=f"rz{name}")
                nc.vector.reciprocal_approx_fast(out=rz[:], in_=z_f[:])
                e_b = p_s1.tile([P, HT, CH], ar_dtype, tag="s1", name=f"eb{name}")
                nc.sync.dma_start(e_b[:], e_d[ch, msl].rearrange("m p c -> p m c"))
                a_sb = p_s1.tile([P, HT, CH], BF16, tag="s1", name=f"a{name}")
                (mult_eng or nc.vector).tensor_mul(
                    out=a_sb[:], in0=e_b[:], in1=rz[:]
                )
                return a_sb

            def z1_src(ch, msl):
                return z1_h[ch // 2][ch % 2, msl].rearrange("m p c -> p m c")

            def z2_src(ch, msl):
                return z2_h[ch // 2][ch % 2, msl].rearrange("m p c -> p m c")

            ps_held = {}

            def u1_pass(ch):
                a1 = [make_a_half(e1_d, z1_src, ch, h, f"1{h}") for h in range(2)]
                tiles = []
                for dt in range(DT):
                    dsl = slice(dt * P, (dt + 1) * P)
                    ps = p_ps.tile([P, CH], F32, tag="ps")
                    for mt in range(NT):
                        nc.tensor.matmul(
                            ps[:], v2[:, mt, dsl], a1[mt // HT][:, mt % HT, :],
                            start=(mt == 0), stop=False,
                        )
                    tiles.append(ps)
                ps_held[ch] = tiles

            def u2_pass(ch):
                a2 = [make_a_half(e2_d, z2_src, ch, h, f"2{h}") for h in range(2)]
                for dt in range(DT):
                    dsl = slice(dt * P, (dt + 1) * P)
                    ps = ps_held[ch][dt]
                    for mt in range(NT):
                        nc.tensor.matmul(
                            ps[:], v1[:, mt, dsl], a2[mt // HT][:, mt % HT, :],
                            start=False, stop=(mt == NT - 1),
                        )
                    xyres = p_small.tile([P, CH], F32, tag="xyres")
                    nc.scalar.dma_start(xyres[:], xyf[dt, ch])
                    ot = p_small.tile([P, CH], F32, tag="ot")
                    nc.vector.tensor_add(out=ot[:], in0=ps[:], in1=xyres[:])
                    nc.scalar.dma_start(out[dt, ch], ot[:])

            u1_pass(0)
            u1_pass(1)
            u2_pass(0)
            u2_pass(1)
            u1_pass(2)
            u1_pass(3)
            u2_pass(2)
            u2_pass(3)

    nc.compile()
    return nc


def _pmajor(a, inner):
    """[O*P, F] -> [P, O, F] partition-major."""
    o = a.shape[0] // inner
    return np.ascontiguousarray(a.reshape(o, inner, a.shape[1]).transpose(1, 0, 2))


def _blocked(a):
    """[D, N] -> [DT, NCH, P, CH] blocked."""
    return np.ascontiguousarray(a.reshape(DT, P, NCH, CH).transpose(0, 2, 1, 3))


def _prep_inputs(inputs):
    import ml_dtypes

    X = np.asarray(inputs["X"], dtype=np.float32)
    Y = np.asarray(inputs["Y"], dtype=np.float32)
    scale = np.float32(1.0 / np.sqrt(D))

    def wT(name, s=np.float32(1.0)):
        w = np.asarray(inputs[f"W_{name}"], dtype=np.float32)
        return _pmajor((w.T * s).astype(ml_dtypes.bfloat16), P)

    def bstripe(name, s=np.float32(1.0)):
        b = np.asarray(inputs[f"b_{name}"], dtype=np.float32) * s
        return np.ascontiguousarray(b.reshape(DT, P).T)

    def bbcast(name):
        b = np.asarray(inputs[f"b_{name}"], dtype=np.float32)
        return np.ascontiguousarray(np.broadcast_to(b, (P, D)))

    shared = {
        "WQ1T": wT("xq", scale),
        "WK1T": wT("xk"),
        "WV1T": wT("xv"),
        "WQ2T": wT("yq", scale),
        "WK2T": wT("yk"),
        "WV2T": wT("yv"),
        "BQ1": bstripe("xq", scale),
        "BK1": bstripe("xk"),
        "BQ2": bstripe("yq", scale),
        "BK2": bstripe("yk"),
        "BV1": bbcast("xv"),
        "BV2": bbcast("yv"),
    }
    in_maps = []
    for c in range(NCORES):
        xt = np.ascontiguousarray(X[c].T)
        yt = np.ascontiguousarray(Y[c].T)
        m = dict(shared)
        m["XYF"] = _blocked(xt + yt)
        m["XTB"] = _pmajor(xt.astype(ml_dtypes.bfloat16), P)
        m["YTB"] = _pmajor(yt.astype(ml_dtypes.bfloat16), P)
        in_maps.append(m)
    return in_maps


def _unblock(ot):
    """[DT, NCH, P, CH] -> [N, D] (transposed back)."""
    return ot.transpose(0, 2, 1, 3).reshape(D, N).T


def kernel(**inputs):
    if "nc" not in _CACHE:
        _CACHE["nc"] = build()
    nc = _CACHE["nc"]
    in_maps = _prep_inputs(inputs)
    res = run_bass_kernel_spmd(
        nc, in_maps, core_ids=list(range(NCORES)), **_CACHE.get("run_kwargs", {})
    )
    _CACHE["last_result"] = res
    out = np.stack(
        [np.ascontiguousarray(_unblock(res.results[c]["OT"])) for c in range(NCORES)]
    )
    return out.astype(np.float32)
